# revision 1
# baseline (speedup 1.0000x reference)
"""CodeGen-style attention block, tensor-parallel over heads on 8 Trainium2 cores.

Strategy (megatron-style):
  - Each core owns 2 of the 16 heads: computes Q/K/V projections for its
    head-slice of w_qkv, runs causal attention for those heads, then applies
    its row-slice of w_out, producing a partial [tokens, H] output.
  - Host sums the 8 partial outputs (the out-proj contraction over heads).

On-chip layout choices:
  - Everything is computed in the "transposed" domain: qT/kT [d, token] come
    straight out of the projection (weights stationary, hidden^T moving), so
    the score matmul scoresT[k, q] = kT_chunk.T @ qT needs no transposes.
  - Softmax runs along the partition (k) axis: exp on ScalarE (mask folded in
    via additive tiles + per-key bias), the denominator via a ones-column
    matmul, normalization deferred to after A@V (per-q reciprocal broadcast
    with a K=1 ones matmul).
  - V is produced transposed like q/k, then flipped to [k, d] with PE
    transposes (needed as the stationary side of A@V).
  - Matmuls use float32r (~1e-4 rel err, ~bf16 speed at N>=256).
  - Causal block-skipping: score/AV work for fully-masked k-chunks is skipped.
"""

import sys
import types
from contextlib import ExitStack

import numpy as np

import concourse.bacc as bacc
import concourse.mybir as mybir
import concourse.tile as tile
from concourse.bass_utils import run_bass_kernel_spmd

# bass_utils imports antenv.axon_hooks when tracing is requested via env;
# provide a no-op stub if the module is absent so a stray BASS_TRACE in the
# environment cannot break execution.
try:
    import antenv.axon_hooks  # noqa: F401
except ImportError:
    _stub = types.ModuleType("antenv.axon_hooks")
    _stub.get_axon_ntff_profile_hook = lambda: None
    _stub.set_axon_ntff_profile_hook = lambda h: None
    sys.modules.setdefault("antenv.axon_hooks", _stub)

F32 = mybir.dt.float32
F32R = mybir.dt.float32r
AF = mybir.ActivationFunctionType

B, S, H = 2, 2048, 4096
N_HEAD, HEAD_DIM, ROT = 16, 256, 64
MAX_POS = 2048
TOK = B * S            # 4096
N_CORES = 8
HPC = N_HEAD // N_CORES  # heads per core = 2
DPC = HPC * HEAD_DIM     # dims per core = 512
NEG = -30000.0

LAST_EXEC_NS = None
_NC_CACHE = []


def _build():
    nc = bacc.Bacc("TRN2", target_bir_lowering=False, debug=False,
                   num_devices=N_CORES)

    # [w, p, hc*256+t]: hsT tiles, per-partition-contiguous
    hst_d = nc.dram_tensor("hst", [16, 128, 32 * 256], F32R, kind="ExternalInput")
    # [oc, p, hc*128+d]: per-core w_qkv column-chunks (q0..3 k0..3 v0..3)
    wqkv_d = nc.dram_tensor("wqkv", [12, 128, 32 * 128], F32R, kind="ExternalInput")
    # [p, c, n]: per-core w_out row-slice
    wout_d = nc.dram_tensor("wout", [128, 4, H], F32R, kind="ExternalInput")
    rope_d = nc.dram_tensor("rope", [128, TOK], F32, kind="ExternalInput")
    rt_d = nc.dram_tensor("rt", [64, 64], F32R, kind="ExternalInput")
    id_d = nc.dram_tensor("ident", [128, 128], F32, kind="ExternalInput")
    onm_d = nc.dram_tensor("onesm", [128, 128], F32R, kind="ExternalInput")
    msk_d = nc.dram_tensor("masks", [128, 4, 512], F32, kind="ExternalInput")
    kb_d = nc.dram_tensor("kb", [128, 32], F32, kind="ExternalInput")
    out_d = nc.dram_tensor("out", [TOK, H], F32, kind="ExternalOutput")

    # phase-1 oc order: v and k first so attention inputs for batch 0 are
    # ready while the q projections still run; q last.
    PASS_OCS = ((8, 9, 10, 11, 4, 5), (6, 7, 0, 1, 2, 3))

    with tile.TileContext(nc) as tc:
        with ExitStack() as st0:
            ec0 = st0.enter_context
            dram_pool = ec0(tc.tile_pool(name="dram", bufs=1, space="DRAM"))
            # per-(oc, batch) intermediates so phase-2 loads only wait on the
            # phase-1 windows they actually read
            qkvT = {}
            for oc in range(12):
                for b in range(2):
                    qkvT[(oc, b)] = dram_pool.tile(
                        [128, 2048], F32R, tag=f"qkvT{oc}_{b}",
                        name=f"qkvT{oc}_{b}")
            # small phase-2 constants loaded up-front (DMA is idle-ish early)
            c2 = ec0(tc.tile_pool(name="p2c", bufs=1))
            msk_sb = c2.tile([128, 4, 512], F32)
            nc.sync.dma_start(msk_sb[:], msk_d[:])
            kb_sb = c2.tile([128, 32], F32)
            nc.sync.dma_start(kb_sb[:], kb_d[:])
            id_sb = c2.tile([128, 128], F32)
            nc.sync.dma_start(id_sb[:], id_d[:])
            onm_sb = c2.tile([128, 128], F32R)
            nc.sync.dma_start(onm_sb[:], onm_d[:])

            # ---------------- Phase 1: QKV projection + rotary ----------------
            with ExitStack() as st1:
                ec = st1.enter_context
                cpool = ec(tc.tile_pool(name="p1c", bufs=1))
                wpool = ec(tc.tile_pool(name="w", bufs=1))
                hpool = ec(tc.tile_pool(name="ht", bufs=2))
                spool = ec(tc.tile_pool(name="stage", bufs=6))
                tpool = ec(tc.tile_pool(name="rott", bufs=4))
                apool = ec(tc.tile_pool(name="acc", bufs=4, space="PSUM"))
                rpool = ec(tc.tile_pool(name="rp", bufs=2, space="PSUM"))
                rope_sb = cpool.tile([128, TOK], F32)
                rt_sb = cpool.tile([64, 64], F32R)

                def load_w(ocs, j0=0):
                    wts = []
                    for j, oc in enumerate(ocs):
                        wt = wpool.tile([128, 32 * 128], F32R, tag=f"w{j0 + j}",
                                        name=f"wt{j0 + j}")
                        nc.sync.dma_start(wt[:], wqkv_d[oc])
                        wts.append(wt)
                    return wts

                def ht_load(w, strips):
                    # strip the transfer so the first H-chunks land (and the
                    # first matmuls start) before the whole 8MB tile arrives
                    t = hpool.tile([128, 32 * 256], F32R, name="ht")
                    step = 32 // strips
                    for s in range(strips):
                        cs = slice(s * step * 256, (s + 1) * step * 256)
                        nc.sync.dma_start(t[:, cs], hst_d[w][:, cs])
                    return t

                wts = load_w(PASS_OCS[0][:1])  # w0 first: first MMs need it
                ht = ht_load(0, 4)
                wts += load_w(PASS_OCS[0][1:], j0=1)
                nc.sync.dma_start(rope_sb[:], rope_d[:])
                nc.sync.dma_start(rt_sb[:], rt_d[:])
                for p, ocs in enumerate(PASS_OCS):
                    if p > 0:
                        wts = next_wts
                        ht = next_ht
                    for w in range(16):
                        if w > 0:
                            ht = next_ht
                        ws = slice(w * 256, (w + 1) * 256)
                        wb, wo = w // 8, (w % 8) * 256
                        for j, oc in enumerate(ocs):
                            if j == 1:
                                # prefetch next window under this one's
                                # compute; strip the early windows so partial
                                # tiles unblock matmuls during the startup
                                # DMA backlog
                                if w < 15:
                                    next_ht = ht_load(w + 1, 2 if w < 4 else 1)
                                elif p + 1 < len(PASS_OCS):
                                    next_ht = ht_load(0, 2)
                            acc = apool.tile([128, 256], F32)
                            for hc in range(32):
                                nc.tensor.matmul(
                                    acc[:],
                                    wts[j][:, hc * 128:(hc + 1) * 128],
                                    ht[:, hc * 256:(hc + 1) * 256],
                                    start=(hc == 0), stop=(hc == 31),
                                )
                            stage = spool.tile([128, 256], F32R)
                            nc.vector.tensor_copy(stage[:], acc[:])
                            if oc in (0, 2, 4, 6):
                                # partial rotary on first 64 dims of this head
                                rp = rpool.tile([64, 256], F32)
                                nc.tensor.matmul(rp[:], rt_sb[:], stage[0:64, :])
                                t1 = tpool.tile([64, 256], F32, tag="t1")
                                nc.vector.tensor_mul(
                                    t1[:], acc[0:64, :], rope_sb[0:64, ws])
                                t2 = tpool.tile([64, 256], F32, tag="t2")
                                nc.vector.tensor_mul(
                                    t2[:], rp[:], rope_sb[64:128, ws])
                                nc.vector.tensor_add(stage[0:64, :], t1[:], t2[:])
                            nc.sync.dma_start(
                                qkvT[(oc, wb)][:, wo:wo + 256], stage[:])
                        if w == 15 and p + 1 < len(PASS_OCS):
                            # issue next pass's weight DMAs under this
                            # window's remaining compute
                            next_wts = load_w(PASS_OCS[p + 1])

            # ---------------- Phase 2: attention + out-proj ----------------
            with ExitStack() as st2:
                ec = st2.enter_context
                c3 = ec(tc.tile_pool(name="p2w", bufs=1))
                kpool = ec(tc.tile_pool(name="kt", bufs=1))
                vtpool = ec(tc.tile_pool(name="vt", bufs=2))
                vhpool = ec(tc.tile_pool(name="vh", bufs=1))
                qpool = ec(tc.tile_pool(name="qq", bufs=2))
                expool = ec(tc.tile_pool(name="ex", bufs=4))
                recpool = ec(tc.tile_pool(name="rec", bufs=2))
                aopool = ec(tc.tile_pool(name="ao", bufs=2))
                ospool = ec(tc.tile_pool(name="os", bufs=3))
                scpool = ec(tc.tile_pool(name="sc", bufs=2, space="PSUM"))
                avpool = ec(tc.tile_pool(name="av", bufs=1, space="PSUM"))
                denpool = ec(tc.tile_pool(name="den", bufs=2, space="PSUM"))
                oppool = ec(tc.tile_pool(name="op", bufs=2, space="PSUM"))
                wout_sb = c3.tile([128, 4, H], F32R)

                def emit_outproj(b, qt, aos):
                    qo = qt * 512
                    for tc_ in range(4):
                        for ht_ in range(8):
                            op = oppool.tile([128, 512], F32, tag="op")
                            for ci, (hl, dc) in enumerate(
                                    ((0, 0), (0, 1), (1, 0), (1, 1))):
                                nc.tensor.matmul(
                                    op[:],
                                    aos[(hl, dc)][:, tc_ * 128:(tc_ + 1) * 128],
                                    wout_sb[:, 2 * hl + dc,
                                            ht_ * 512:(ht_ + 1) * 512],
                                    start=(ci == 0), stop=(ci == 3))
                            os_ = ospool.tile([128, 512], F32)
                            nc.vector.tensor_copy(os_[:], op[:])
                            r0 = b * 2048 + qo + tc_ * 128
                            nc.sync.dma_start(
                                out_d[r0:r0 + 128, ht_ * 512:(ht_ + 1) * 512],
                                os_[:])

                pending = None
                for b in range(2):
                    kts = {}
                    vhs = {}
                    for hl in range(2):
                        vh = vhpool.tile([128, 16 * 256], F32R, tag=f"vh{hl}")
                        for dc in range(2):
                            vt = vtpool.tile([128, 2048], F32)
                            nc.sync.dma_start(
                                vt[:], qkvT[(8 + 2 * hl + dc, b)][:].bitcast(F32))
                            for kc in range(16):
                                tp = oppool.tile([128, 128], F32, tag="op")
                                nc.tensor.transpose(
                                    tp[:], vt[:, kc * 128:(kc + 1) * 128], id_sb[:])
                                nc.vector.tensor_copy(
                                    vh[:, kc * 256 + dc * 128:
                                       kc * 256 + (dc + 1) * 128], tp[:])
                        vhs[hl] = vh
                        for dc in range(2):
                            kt = kpool.tile([128, 2048], F32R, tag=f"kt{hl}{dc}")
                            nc.sync.dma_start(kt[:], qkvT[(4 + 2 * hl + dc, b)][:])
                            kts[(hl, dc)] = kt
                    for qt in range(4):
                        nkc = 4 * qt + 4  # causal: k-chunks beyond are all-masked
                        qo = qt * 512
                        aos = {}
                        for hl in range(2):
                            qs = []
                            for dc in range(2):
                                q = qpool.tile([128, 512], F32R, tag=f"q{dc}")
                                nc.sync.dma_start(
                                    q[:], qkvT[(2 * hl + dc, b)][:, qo:qo + 512])
                                qs.append(q)
                            if b == 0 and qt == 0 and hl == 0:
                                # out-proj weights are first needed one
                                # qt-block in; issue this 8MB DMA after the
                                # first attention inputs, not before
                                nc.sync.dma_start(wout_sb[:], wout_d[:])
                            av0 = avpool.tile([128, 512], F32, tag="av0")
                            av1 = avpool.tile([128, 512], F32, tag="av1")
                            den = denpool.tile([128, 512], F32)
                            for kc in range(nkc):
                                sc = scpool.tile([128, 512], F32)
                                nc.tensor.matmul(
                                    sc[:], kts[(hl, 0)][:, kc * 128:(kc + 1) * 128],
                                    qs[0][:], start=True, stop=False)
                                nc.tensor.matmul(
                                    sc[:], kts[(hl, 1)][:, kc * 128:(kc + 1) * 128],
                                    qs[1][:], start=False, stop=True)
                                if kc >= 4 * qt:
                                    nc.vector.tensor_add(
                                        sc[:], sc[:], msk_sb[:, kc - 4 * qt, :])
                                ex = expool.tile([128, 512], F32R)
                                nc.scalar.activation(
                                    ex[:], sc[:], AF.Exp, scale=1.0 / 16.0,
                                    bias=kb_sb[:, b * 16 + kc:b * 16 + kc + 1])
                                nc.tensor.matmul(
                                    av0[:], vhs[hl][:, kc * 256:kc * 256 + 128],
                                    ex[:], start=(kc == 0), stop=(kc == nkc - 1))
                                nc.tensor.matmul(
                                    av1[:], vhs[hl][:, kc * 256 + 128:kc * 256 + 256],
                                    ex[:], start=(kc == 0), stop=(kc == nkc - 1))
                                # denominator, pre-broadcast across partitions:
                                # ones[128,128].T @ ex = colsum replicated 128x
                                nc.tensor.matmul(
                                    den[:], onm_sb[:], ex[:],
                                    start=(kc == 0), stop=(kc == nkc - 1))
                            # fast av-bank evacuation on ScalarE (DVE's in-order queue is
                            # occupied by the ~3.4us reciprocal); den keeps its
                            # bank through the reciprocal (bufs=2 covers it)
                            avs = []
                            for dc, av in ((0, av0), (1, av1)):
                                avc = aopool.tile([128, 512], F32, bufs=1,
                                                  tag=f"avs{hl}{dc}", name="avc")
                                nc.scalar.copy(avc[:], av[:])
                                avs.append(avc)
                            rec = recpool.tile([128, 512], F32, tag="rec", bufs=1)
                            nc.vector.reciprocal(rec[:], den[:])
                            for dc in range(2):
                                ao = aopool.tile([128, 512], F32R, tag=f"ao{hl}{dc}")
                                nc.vector.tensor_mul(ao[:], avs[dc][:], rec[:])
                                aos[(hl, dc)] = ao
                        # software pipeline: emit the PREVIOUS block's out-proj
                        # here so its matmuls sit behind this block's attention
                        # in PE program order and never wait on normalization
                        if pending is not None:
                            emit_outproj(*pending)
                        pending = (b, qt, aos)
                emit_outproj(*pending)
    nc.compile()
    return nc


def _get_nc():
    if not _NC_CACHE:
        _NC_CACHE.append(_build())
    return _NC_CACHE[0]


def _host_prep(hidden_states, position_ids, attention_mask, w_qkv, w_out):
    hid = np.ascontiguousarray(np.asarray(hidden_states, np.float32)).reshape(TOK, H)
    w_qkv = np.asarray(w_qkv, np.float32)
    w_out = np.asarray(w_out, np.float32)
    pos = np.asarray(position_ids).astype(np.int64)
    am = np.asarray(attention_mask).reshape(B, S).astype(bool)

    # hsT tiles [w, p, hc*256+t]
    hst = np.ascontiguousarray(
        hid.reshape(16, 256, 32, 128).transpose(0, 3, 2, 1)).reshape(16, 128, 32 * 256)

    # rotary tables, matching reference.create_sinusoidal_positions
    inv_freq = 1.0 / 10000 ** (np.arange(0, ROT, 2) / ROT)
    si = np.einsum('i,j->ij', np.arange(MAX_POS), inv_freq).astype('float32')
    emb = np.concatenate([np.sin(si), np.cos(si)], axis=-1)  # [2048, 64]
    sincos = emb[pos]                    # [B, S, 64]
    sin_rep = np.repeat(sincos[..., :ROT // 2], 2, axis=2)   # [B, S, 64]
    cos_rep = np.repeat(sincos[..., ROT // 2:], 2, axis=2)
    rope = np.empty((128, TOK), np.float32)
    rope[0:64] = cos_rep.reshape(TOK, 64).T
    rope[64:128] = sin_rep.reshape(TOK, 64).T

    rt = np.zeros((64, 64), np.float32)
    rt[np.arange(1, 64, 2), np.arange(0, 64, 2)] = -1.0
    rt[np.arange(0, 64, 2), np.arange(1, 64, 2)] = 1.0

    ident = np.eye(128, dtype=np.float32)
    onesm = np.ones((128, 128), np.float32)

    p_idx = np.arange(128)[:, None, None]
    i_idx = np.arange(4)[None, :, None]
    q_idx = np.arange(512)[None, None, :]
    masks = np.where(p_idx + i_idx * 128 <= q_idx, 0.0, NEG).astype(np.float32)

    kb = np.where(am.reshape(B, 16, 128), 0.0, NEG).astype(
        np.float32).transpose(2, 0, 1).reshape(128, 32)
    kb = np.ascontiguousarray(kb)

    shared = dict(hst=hst, rope=rope, rt=rt, ident=ident, onesm=onesm,
                  masks=masks, kb=kb)

    in_maps = []
    for c in range(N_CORES):
        cols = []
        for part in (0, 2, 1):  # fused layout per mp-group is (query, value, key)
            for hl in range(HPC):
                h = HPC * c + hl
                base = (h // 4) * 3072 + part * 1024 + (h % 4) * 256
                cols.append(np.arange(base, base + 256))
        cols = np.concatenate(cols)  # [1536] = q(512) | k(512) | v(512)
        wslice = w_qkv[:, cols]      # [4096, 1536]
        wqkv_prep = np.ascontiguousarray(
            wslice.reshape(32, 128, 12, 128).transpose(2, 1, 0, 3)
        ).reshape(12, 128, 32 * 128)
        wout_prep = np.ascontiguousarray(
            w_out[c * DPC:(c + 1) * DPC, :].reshape(4, 128, H).transpose(1, 0, 2))
        in_maps.append(dict(shared, wqkv=wqkv_prep, wout=wout_prep))
    return in_maps


def kernel(hidden_states, position_ids, attention_mask, w_qkv, w_out):
    global LAST_EXEC_NS
    nc = _get_nc()
    in_maps = _host_prep(hidden_states, position_ids, attention_mask,
                         w_qkv, w_out)
    res = run_bass_kernel_spmd(nc, in_maps, core_ids=list(range(N_CORES)))
    LAST_EXEC_NS = res.exec_time_ns
    out = res.results[0]["out"].astype(np.float32)
    for c in range(1, N_CORES):
        out = out + res.results[c]["out"]
    return out.reshape(B, S, H)



# revision 2
# speedup vs baseline: 1.3932x; 1.3932x over previous
"""CodeGen-style attention block, tensor-parallel over heads on 8 Trainium2 cores.

Strategy (megatron-style):
  - Each core owns 2 of the 16 heads: computes Q/K/V projections for its
    head-slice of w_qkv, runs causal attention for those heads, then applies
    its row-slice of w_out, producing a partial [tokens, H] output.
  - Host sums the 8 partial outputs (the out-proj contraction over heads).

v2 notes (vs the fp32r baseline):
  - All matmuls are fp16 (fp32 PSUM accumulate). On HW, fp32r moving operands
    stream at ~0.58 ns/col (bandwidth-limited) and fp32 LDWEIGHTS skips FWL;
    fp16 streams at the full 1 col/cycle rate and halves every DMA transfer.
  - Single pass over the hidden states: all 12 w_qkv column-chunks stay
    resident in SBUF (fp16 halves their footprint), so hidden is read once.
  - V is projected directly into [token, dim] layout (hidden chunks
    stationary, w_v moving) - no PE transposes / DVE copies in phase 2.
  - Attention inner loop is software-pipelined: scores for chunk kc+1 are
    issued before AV of chunk kc, so the PE never waits on ScalarE's exp.
  - Causal block-skipping: score/AV work for fully-masked k-chunks is skipped.
"""

import sys
import types
from contextlib import ExitStack

import numpy as np

import concourse.bacc as bacc
import concourse.mybir as mybir
import concourse.tile as tile
from concourse.bass_utils import run_bass_kernel_spmd

# bass_utils imports antenv.axon_hooks when tracing is requested via env;
# provide a no-op stub if the module is absent so a stray BASS_TRACE in the
# environment cannot break execution.
try:
    import antenv.axon_hooks  # noqa: F401
except ImportError:
    _stub = types.ModuleType("antenv.axon_hooks")
    _stub.get_axon_ntff_profile_hook = lambda: None
    _stub.set_axon_ntff_profile_hook = lambda h: None
    sys.modules.setdefault("antenv.axon_hooks", _stub)

F32 = mybir.dt.float32
F16 = mybir.dt.float16
AF = mybir.ActivationFunctionType

B, S, H = 2, 2048, 4096
N_HEAD, HEAD_DIM, ROT = 16, 256, 64
MAX_POS = 2048
TOK = B * S            # 4096
N_CORES = 8
HPC = N_HEAD // N_CORES  # heads per core = 2
DPC = HPC * HEAD_DIM     # dims per core = 512
NEG = -30000.0

LAST_EXEC_NS = None
_NC_CACHE = []


def _build():
    nc = bacc.Bacc("TRN2", target_bir_lowering=False, debug=False,
                   num_devices=N_CORES)

    # [w, p, hc*512+t]: hsT window tiles (512 tokens each), per-partition-contiguous
    hst_d = nc.dram_tensor("hst", [8, 128, 32 * 512], F16, kind="ExternalInput")
    # [oc, p, hc*128+d]: per-core w_qkv column-chunks for q (oc 0-3), k (oc 4-7)
    wqkv_d = nc.dram_tensor("wqkv", [8, 128, 32 * 128], F16, kind="ExternalInput")
    # [p, hc*512+v]: per-core w_v slice, hidden-chunk-major (moving operand)
    wv_d = nc.dram_tensor("wv", [128, 32 * 512], F16, kind="ExternalInput")
    # [p, c, n]: per-core w_out row-slice
    wout_d = nc.dram_tensor("wout", [128, 4, H], F16, kind="ExternalInput")
    rope_d = nc.dram_tensor("rope", [128, TOK], F32, kind="ExternalInput")
    rt_d = nc.dram_tensor("rt", [64, 64], F16, kind="ExternalInput")
    onm_d = nc.dram_tensor("onesm", [128, 128], F16, kind="ExternalInput")
    msk_d = nc.dram_tensor("masks", [128, 4, 512], F32, kind="ExternalInput")
    kb_d = nc.dram_tensor("kb", [128, 32], F32, kind="ExternalInput")
    out_d = nc.dram_tensor("out", [TOK, H], F16, kind="ExternalOutput")

    # per-window oc order: k first, then q (phase-2 needs k/v before q), v last
    # interleaved via tc; v goes first so vh windows land earliest.
    QK_OCS = (4, 5, 6, 7, 0, 1, 2, 3)

    with tile.TileContext(nc) as tc:
        with ExitStack() as st0:
            ec0 = st0.enter_context
            dram_pool = ec0(tc.tile_pool(name="dram", bufs=1, space="DRAM"))
            # per-(oc, batch) intermediates so phase-2 loads only wait on the
            # phase-1 windows they actually read
            qkvT = {}
            for oc in range(8):
                for b in range(2):
                    qkvT[(oc, b)] = dram_pool.tile(
                        [128, 2048], F16, tag=f"qkvT{oc}_{b}",
                        name=f"qkvT{oc}_{b}")
            vh_dram = {}
            for b in range(2):
                vh_dram[b] = dram_pool.tile(
                    [128, 16 * 512], F16, tag=f"vh{b}", name=f"vh{b}")
            # small phase-2 constants loaded up-front (DMA is idle-ish early)
            c2 = ec0(tc.tile_pool(name="p2c", bufs=1))
            msk_sb = c2.tile([128, 4, 512], F32)
            nc.sync.dma_start(msk_sb[:], msk_d[:])
            kb_sb = c2.tile([128, 32], F32)
            nc.sync.dma_start(kb_sb[:], kb_d[:])
            onm_sb = c2.tile([128, 128], F16)
            nc.sync.dma_start(onm_sb[:], onm_d[:])

            # ---------------- Phase 1: QKV projection + rotary ----------------
            with ExitStack() as st1:
                ec = st1.enter_context
                cpool = ec(tc.tile_pool(name="p1c", bufs=1))
                wpool = ec(tc.tile_pool(name="w", bufs=1))
                hpool = ec(tc.tile_pool(name="ht", bufs=2))
                spool = ec(tc.tile_pool(name="stage", bufs=6))
                tpool = ec(tc.tile_pool(name="rott", bufs=4))
                apool = ec(tc.tile_pool(name="acc", bufs=4, space="PSUM"))
                rpool = ec(tc.tile_pool(name="rp", bufs=2, space="PSUM"))
                rope_sb = cpool.tile([128, TOK], F32)
                rt_sb = cpool.tile([64, 64], F16)

                def ht_load(w, strips):
                    # strip the transfer so the first H-chunks land (and the
                    # first matmuls start) before the whole 4MB tile arrives
                    t = hpool.tile([128, 32 * 512], F16, name="ht")
                    step = 32 // strips
                    for s in range(strips):
                        cs = slice(s * step * 512, (s + 1) * step * 512)
                        nc.sync.dma_start(t[:, cs], hst_d[w][:, cs])
                    return t

                # wv first: the first MMs of window 0 are the v projection
                wv_sb = wpool.tile([128, 32 * 512], F16, name="wv")
                nc.sync.dma_start(wv_sb[:], wv_d[:])
                ht = ht_load(0, 4)
                wts = []
                for j, oc in enumerate(QK_OCS):
                    wt = wpool.tile([128, 32 * 128], F16, tag=f"w{j}",
                                    name=f"wt{j}")
                    nc.sync.dma_start(wt[:], wqkv_d[oc])
                    wts.append(wt)
                nc.sync.dma_start(rope_sb[:], rope_d[:])
                nc.sync.dma_start(rt_sb[:], rt_d[:])

                for w in range(8):
                    if w > 0:
                        ht = next_ht
                    b, wo = w // 4, (w % 4) * 512
                    ws = slice(w * 512, (w + 1) * 512)
                    # --- v projection: hidden chunks stationary, w_v moving;
                    # output lands directly as [token, vdim]
                    for tc_ in range(4):
                        if tc_ == 1 and w < 7:
                            # prefetch next window under this one's compute
                            next_ht = ht_load(w + 1, 2 if w < 2 else 1)
                        acc = apool.tile([128, 512], F32)
                        for hc in range(32):
                            nc.tensor.matmul(
                                acc[:],
                                ht[:, hc * 512 + tc_ * 128:
                                   hc * 512 + (tc_ + 1) * 128],
                                wv_sb[:, hc * 512:(hc + 1) * 512],
                                start=(hc == 0), stop=(hc == 31),
                            )
                        stage = spool.tile([128, 512], F16)
                        nc.scalar.copy(stage[:], acc[:])
                        kc = (w % 4) * 4 + tc_
                        nc.sync.dma_start(
                            vh_dram[b][:, kc * 512:(kc + 1) * 512], stage[:])
                    # --- q/k projections: weights stationary, hidden moving;
                    # output lands transposed as [dim, token]
                    for j, oc in enumerate(QK_OCS):
                        acc = apool.tile([128, 512], F32)
                        for hc in range(32):
                            nc.tensor.matmul(
                                acc[:],
                                wts[j][:, hc * 128:(hc + 1) * 128],
                                ht[:, hc * 512:(hc + 1) * 512],
                                start=(hc == 0), stop=(hc == 31),
                            )
                        stage = spool.tile([128, 512], F16)
                        nc.scalar.copy(stage[:], acc[:])
                        if oc in (0, 2, 4, 6):
                            # partial rotary on first 64 dims of this head
                            rp = rpool.tile([64, 512], F32)
                            nc.tensor.matmul(rp[:], rt_sb[:], stage[0:64, :])
                            t1 = tpool.tile([64, 512], F32, tag="t1")
                            nc.vector.tensor_mul(
                                t1[:], acc[0:64, :], rope_sb[0:64, ws])
                            t2 = tpool.tile([64, 512], F32, tag="t2")
                            nc.vector.tensor_mul(
                                t2[:], rp[:], rope_sb[64:128, ws])
                            nc.vector.tensor_add(stage[0:64, :], t1[:], t2[:])
                        nc.sync.dma_start(
                            qkvT[(oc, b)][:, wo:wo + 512], stage[:])

            # ---------------- Phase 2: attention + out-proj ----------------
            with ExitStack() as st2:
                ec = st2.enter_context
                c3 = ec(tc.tile_pool(name="p2w", bufs=1))
                kpool = ec(tc.tile_pool(name="kt", bufs=1))
                vhpool = ec(tc.tile_pool(name="vh", bufs=1))
                qpool = ec(tc.tile_pool(name="qq", bufs=2))
                expool = ec(tc.tile_pool(name="ex", bufs=4))
                recpool = ec(tc.tile_pool(name="rec", bufs=2))
                aopool = ec(tc.tile_pool(name="ao", bufs=2))
                ospool = ec(tc.tile_pool(name="os", bufs=4))
                scpool = ec(tc.tile_pool(name="sc", bufs=2, space="PSUM"))
                avpool = ec(tc.tile_pool(name="av", bufs=1, space="PSUM"))
                denpool = ec(tc.tile_pool(name="den", bufs=2, space="PSUM"))
                oppool = ec(tc.tile_pool(name="op", bufs=2, space="PSUM"))
                wout_sb = c3.tile([128, 4, H], F16)

                def emit_outproj(b, qt, aos):
                    qo = qt * 512
                    for tc_ in range(4):
                        for ht_ in range(8):
                            op = oppool.tile([128, 512], F32, tag="op")
                            for ci, (hl, dc) in enumerate(
                                    ((0, 0), (0, 1), (1, 0), (1, 1))):
                                nc.tensor.matmul(
                                    op[:],
                                    aos[(hl, dc)][:, tc_ * 128:(tc_ + 1) * 128],
                                    wout_sb[:, 2 * hl + dc,
                                            ht_ * 512:(ht_ + 1) * 512],
                                    start=(ci == 0), stop=(ci == 3))
                            os_ = ospool.tile([128, 512], F16)
                            # split PSUM evacuation across both copy engines
                            if ht_ % 2 == 0:
                                nc.scalar.copy(os_[:], op[:])
                            else:
                                nc.vector.tensor_copy(os_[:], op[:])
                            r0 = b * 2048 + qo + tc_ * 128
                            nc.sync.dma_start(
                                out_d[r0:r0 + 128, ht_ * 512:(ht_ + 1) * 512],
                                os_[:])

                pending = None
                for b in range(2):
                    vh = vhpool.tile([128, 16 * 512], F16, tag="vha")
                    nc.sync.dma_start(vh[:], vh_dram[b][:])
                    kts = {}
                    for hl in range(2):
                        for dc in range(2):
                            kt = kpool.tile([128, 2048], F16, tag=f"kt{hl}{dc}")
                            nc.sync.dma_start(kt[:], qkvT[(4 + 2 * hl + dc, b)][:])
                            kts[(hl, dc)] = kt
                    for qt in range(4):
                        nkc = 4 * qt + 4  # causal: k-chunks beyond are all-masked
                        qo = qt * 512
                        aos = {}
                        for hl in range(2):
                            qs = []
                            for dc in range(2):
                                q = qpool.tile([128, 512], F16, tag=f"q{dc}")
                                nc.sync.dma_start(
                                    q[:], qkvT[(2 * hl + dc, b)][:, qo:qo + 512])
                                qs.append(q)
                            if b == 0 and qt == 0 and hl == 0:
                                # out-proj weights are first needed one
                                # qt-block in; issue this 16MB DMA after the
                                # first attention inputs, not before
                                nc.sync.dma_start(wout_sb[:], wout_d[:])
                            av0 = avpool.tile([128, 512], F32, tag="av0")
                            av1 = avpool.tile([128, 512], F32, tag="av1")
                            den = denpool.tile([128, 512], F32)

                            def emit_av(kc, ex):
                                nc.tensor.matmul(
                                    av0[:],
                                    vh[:, kc * 512 + hl * 256:
                                       kc * 512 + hl * 256 + 128],
                                    ex[:], start=(kc == 0), stop=(kc == nkc - 1))
                                nc.tensor.matmul(
                                    av1[:],
                                    vh[:, kc * 512 + hl * 256 + 128:
                                       kc * 512 + hl * 256 + 256],
                                    ex[:], start=(kc == 0), stop=(kc == nkc - 1))
                                # denominator, pre-broadcast across partitions:
                                # ones[128,128].T @ ex = colsum replicated 128x
                                nc.tensor.matmul(
                                    den[:], onm_sb[:], ex[:],
                                    start=(kc == 0), stop=(kc == nkc - 1))

                            prev = None
                            for kc in range(nkc):
                                sc = scpool.tile([128, 512], F32)
                                nc.tensor.matmul(
                                    sc[:], kts[(hl, 0)][:, kc * 128:(kc + 1) * 128],
                                    qs[0][:], start=True, stop=False)
                                nc.tensor.matmul(
                                    sc[:], kts[(hl, 1)][:, kc * 128:(kc + 1) * 128],
                                    qs[1][:], start=False, stop=True)
                                if kc >= 4 * qt:
                                    nc.vector.tensor_add(
                                        sc[:], sc[:], msk_sb[:, kc - 4 * qt, :])
                                # software pipeline: AV for the PREVIOUS chunk
                                # is emitted after this chunk's score matmuls,
                                # hiding the exp latency from the PE stream
                                if prev is not None:
                                    emit_av(*prev)
                                ex = expool.tile([128, 512], F16)
                                nc.scalar.activation(
                                    ex[:], sc[:], AF.Exp, scale=1.0 / 16.0,
                                    bias=kb_sb[:, b * 16 + kc:b * 16 + kc + 1])
                                prev = (kc, ex)
                            emit_av(*prev)
                            # fast av-bank evacuation on ScalarE (DVE's in-order
                            # queue runs the ~4us reciprocal); den keeps its
                            # bank through the reciprocal (bufs=2 covers it)
                            avs = []
                            for dc, av in ((0, av0), (1, av1)):
                                avc = aopool.tile([128, 512], F32, bufs=1,
                                                  tag=f"avs{hl}{dc}", name="avc")
                                nc.scalar.copy(avc[:], av[:])
                                avs.append(avc)
                            rec = recpool.tile([128, 512], F32, tag="rec", bufs=1)
                            nc.vector.reciprocal(rec[:], den[:])
                            for dc in range(2):
                                ao = aopool.tile([128, 512], F16, tag=f"ao{hl}{dc}")
                                nc.vector.tensor_mul(ao[:], avs[dc][:], rec[:])
                                aos[(hl, dc)] = ao
                        # software pipeline: emit the PREVIOUS block's out-proj
                        # here so its matmuls sit behind this block's attention
                        # in PE program order and never wait on normalization
                        if pending is not None:
                            emit_outproj(*pending)
                        pending = (b, qt, aos)
                emit_outproj(*pending)
    nc.compile()
    return nc


def _get_nc():
    if not _NC_CACHE:
        _NC_CACHE.append(_build())
    return _NC_CACHE[0]


def _host_prep(hidden_states, position_ids, attention_mask, w_qkv, w_out):
    hid = np.ascontiguousarray(np.asarray(hidden_states, np.float32)).reshape(TOK, H)
    w_qkv = np.asarray(w_qkv, np.float32)
    w_out = np.asarray(w_out, np.float32)
    pos = np.asarray(position_ids).astype(np.int64)
    am = np.asarray(attention_mask).reshape(B, S).astype(bool)

    # hsT window tiles [w, p, hc*512+t]
    hst = np.ascontiguousarray(
        hid.reshape(8, 512, 32, 128).transpose(0, 3, 2, 1)
    ).reshape(8, 128, 32 * 512).astype(np.float16)

    # rotary tables, matching reference.create_sinusoidal_positions
    inv_freq = 1.0 / 10000 ** (np.arange(0, ROT, 2) / ROT)
    si = np.einsum('i,j->ij', np.arange(MAX_POS), inv_freq).astype('float32')
    emb = np.concatenate([np.sin(si), np.cos(si)], axis=-1)  # [2048, 64]
    sincos = emb[pos]                    # [B, S, 64]
    sin_rep = np.repeat(sincos[..., :ROT // 2], 2, axis=2)   # [B, S, 64]
    cos_rep = np.repeat(sincos[..., ROT // 2:], 2, axis=2)
    rope = np.empty((128, TOK), np.float32)
    rope[0:64] = cos_rep.reshape(TOK, 64).T
    rope[64:128] = sin_rep.reshape(TOK, 64).T

    rt = np.zeros((64, 64), np.float16)
    rt[np.arange(1, 64, 2), np.arange(0, 64, 2)] = -1.0
    rt[np.arange(0, 64, 2), np.arange(1, 64, 2)] = 1.0

    onesm = np.ones((128, 128), np.float16)

    p_idx = np.arange(128)[:, None, None]
    i_idx = np.arange(4)[None, :, None]
    q_idx = np.arange(512)[None, None, :]
    masks = np.where(p_idx + i_idx * 128 <= q_idx, 0.0, NEG).astype(np.float32)

    kb = np.where(am.reshape(B, 16, 128), 0.0, NEG).astype(
        np.float32).transpose(2, 0, 1).reshape(128, 32)
    kb = np.ascontiguousarray(kb)

    shared = dict(hst=hst, rope=rope, rt=rt, onesm=onesm, masks=masks, kb=kb)

    in_maps = []
    for c in range(N_CORES):
        qk_cols = []
        v_cols = []
        # fused layout per mp-group is (query, value, key)
        for part, dest in ((0, qk_cols), (2, qk_cols), (1, v_cols)):
            for hl in range(HPC):
                h = HPC * c + hl
                base = (h // 4) * 3072 + part * 1024 + (h % 4) * 256
                dest.append(np.arange(base, base + 256))
        qk_cols = np.concatenate(qk_cols)  # [1024] = q(512) | k(512)
        wslice = w_qkv[:, qk_cols]         # [4096, 1024]
        wqkv_prep = np.ascontiguousarray(
            wslice.reshape(32, 128, 8, 128).transpose(2, 1, 0, 3)
        ).reshape(8, 128, 32 * 128).astype(np.float16)
        v_cols = np.concatenate(v_cols)    # [512]
        wv_prep = np.ascontiguousarray(
            w_qkv[:, v_cols].reshape(32, 128, 512).transpose(1, 0, 2)
        ).reshape(128, 32 * 512).astype(np.float16)
        wout_prep = np.ascontiguousarray(
            w_out[c * DPC:(c + 1) * DPC, :].reshape(4, 128, H).transpose(1, 0, 2)
        ).astype(np.float16)
        in_maps.append(dict(shared, wqkv=wqkv_prep, wv=wv_prep, wout=wout_prep))
    return in_maps


def kernel(hidden_states, position_ids, attention_mask, w_qkv, w_out):
    global LAST_EXEC_NS
    nc = _get_nc()
    in_maps = _host_prep(hidden_states, position_ids, attention_mask,
                         w_qkv, w_out)
    res = run_bass_kernel_spmd(nc, in_maps, core_ids=list(range(N_CORES)))
    LAST_EXEC_NS = res.exec_time_ns
    out = res.results[0]["out"].astype(np.float32)
    for c in range(1, N_CORES):
        out = out + res.results[c]["out"].astype(np.float32)
    return out.reshape(B, S, H)


# revision 3
# speedup vs baseline: 1.4126x; 1.0139x over previous
"""CodeGen-style attention block, tensor-parallel over heads on 8 Trainium2 cores.

Strategy (megatron-style):
  - Each core owns 2 of the 16 heads: computes Q/K/V projections for its
    head-slice of w_qkv, runs causal attention for those heads, then applies
    its row-slice of w_out, producing a partial [tokens, H] output.
  - Host sums the 8 partial outputs (the out-proj contraction over heads).

v3 notes:
  - All matmuls are fp16 (fp32 PSUM accumulate). On HW, fp32r moving operands
    stream at ~0.55 ns/col (bandwidth-limited) and fp32 LDWEIGHTS skips FWL;
    fp16 streams at the full 1 col/cycle rate and halves every DMA transfer.
  - Single pass over the hidden states: all 12 w_qkv column-chunks stay
    resident in SBUF (fp16 halves their footprint), so hidden is read once.
  - V is projected directly into [token, dim] layout (hidden chunks
    stationary, w_v moving) - no PE transposes / DVE copies in phase 2.
  - K for batch 0 is written straight into resident SBUF tiles by phase 1
    (no DRAM round-trip), so phase 2 starts with only a q-tile load pending.
  - Attention inner loop is software-pipelined: scores for chunk kc+1 are
    issued before AV of chunk kc, so the PE never waits on ScalarE's exp.
  - The softmax denominator accumulates on VectorE (one tensor_add per
    k-chunk) with a single partition-reduce matmul per block, replacing the
    per-chunk ones-matmul on the PE.
  - w_out streams in 8 column chunks so the first out-proj block only waits
    on 2MB, not 16MB.
  - Causal block-skipping: score/AV work for fully-masked k-chunks is skipped.
"""

import sys
import types
from contextlib import ExitStack

import numpy as np

import concourse.bacc as bacc
import concourse.mybir as mybir
import concourse.tile as tile
from concourse.bass_utils import run_bass_kernel_spmd

# bass_utils imports antenv.axon_hooks when tracing is requested via env;
# provide a no-op stub if the module is absent so a stray BASS_TRACE in the
# environment cannot break execution.
try:
    import antenv.axon_hooks  # noqa: F401
except ImportError:
    _stub = types.ModuleType("antenv.axon_hooks")
    _stub.get_axon_ntff_profile_hook = lambda: None
    _stub.set_axon_ntff_profile_hook = lambda h: None
    sys.modules.setdefault("antenv.axon_hooks", _stub)

F32 = mybir.dt.float32
F16 = mybir.dt.float16
AF = mybir.ActivationFunctionType

B, S, H = 2, 2048, 4096
N_HEAD, HEAD_DIM, ROT = 16, 256, 64
MAX_POS = 2048
TOK = B * S            # 4096
N_CORES = 8
HPC = N_HEAD // N_CORES  # heads per core = 2
DPC = HPC * HEAD_DIM     # dims per core = 512
NEG = -30000.0

LAST_EXEC_NS = None
_NC_CACHE = []


def _build():
    nc = bacc.Bacc("TRN2", target_bir_lowering=False, debug=False,
                   num_devices=N_CORES)

    # [w, p, hc*512+t]: hsT window tiles (512 tokens each), per-partition-contiguous
    hst_d = nc.dram_tensor("hst", [8, 128, 32 * 512], F16, kind="ExternalInput")
    # [oc, p, hc*128+d]: per-core w_qkv column-chunks for q (oc 0-3), k (oc 4-7)
    wqkv_d = nc.dram_tensor("wqkv", [8, 128, 32 * 128], F16, kind="ExternalInput")
    # [p, hc*512+v]: per-core w_v slice, hidden-chunk-major (moving operand)
    wv_d = nc.dram_tensor("wv", [128, 32 * 512], F16, kind="ExternalInput")
    # [p, c, n]: per-core w_out row-slice
    wout_d = nc.dram_tensor("wout", [128, 4, H], F16, kind="ExternalInput")
    rope_d = nc.dram_tensor("rope", [128, TOK], F16, kind="ExternalInput")
    rt_d = nc.dram_tensor("rt", [64, 64], F16, kind="ExternalInput")
    onm_d = nc.dram_tensor("onesm", [128, 128], F32, kind="ExternalInput")
    msk_d = nc.dram_tensor("masks", [128, 4, 512], F16, kind="ExternalInput")
    kb_d = nc.dram_tensor("kb", [128, 32], F32, kind="ExternalInput")
    out_d = nc.dram_tensor("out", [TOK, H], F16, kind="ExternalOutput")

    # per-window oc order: k first (its outputs land in resident SBUF tiles
    # for b=0 and phase 2 reads k/v before q), then q, then v.
    K_OCS = (4, 5, 6, 7)
    Q_OCS = (0, 1, 2, 3)

    with tile.TileContext(nc) as tc:
        with ExitStack() as st0:
            ec0 = st0.enter_context
            dram_pool = ec0(tc.tile_pool(name="dram", bufs=1, space="DRAM"))
            # per-(oc, batch) intermediates so phase-2 loads only wait on the
            # phase-1 windows they actually read. b=0 k skips DRAM entirely.
            qkvT = {}
            for oc in range(8):
                for b in range(2):
                    if oc >= 4 and b == 0:
                        continue
                    qkvT[(oc, b)] = dram_pool.tile(
                        [128, 2048], F16, tag=f"qkvT{oc}_{b}",
                        name=f"qkvT{oc}_{b}")
            vh_dram = {}
            for b in range(2):
                vh_dram[b] = dram_pool.tile(
                    [128, 16 * 512], F16, tag=f"vh{b}", name=f"vh{b}")
            # resident k tiles: phase 1 writes b=0 directly, phase 2 reloads
            # them from DRAM for b=1
            kpool = ec0(tc.tile_pool(name="kt", bufs=1))
            kts = {}
            for hl in range(2):
                for dc in range(2):
                    kts[(hl, dc)] = kpool.tile(
                        [128, 2048], F16, tag=f"kt{hl}{dc}", name=f"kt{hl}{dc}")

            # ---------------- Phase 1: QKV projection + rotary ----------------
            with ExitStack() as st1:
                ec = st1.enter_context
                cpool = ec(tc.tile_pool(name="p1c", bufs=1))
                wpool = ec(tc.tile_pool(name="w", bufs=1))
                hpool = ec(tc.tile_pool(name="ht", bufs=2))
                spool = ec(tc.tile_pool(name="stage", bufs=4))
                tpool = ec(tc.tile_pool(name="rott", bufs=4))
                apool = ec(tc.tile_pool(name="acc", bufs=4, space="PSUM"))
                rpool = ec(tc.tile_pool(name="rp", bufs=2, space="PSUM"))
                rope_sb = cpool.tile([128, TOK], F16)
                rt_sb = cpool.tile([64, 64], F16)

                def ht_load(w, strips):
                    # strip the transfer so the first H-chunks land (and the
                    # first matmuls start) before the whole 4MB tile arrives
                    t = hpool.tile([128, 32 * 512], F16, name="ht")
                    step = 32 // strips
                    for s in range(strips):
                        cs = slice(s * step * 512, (s + 1) * step * 512)
                        nc.sync.dma_start(t[:, cs], hst_d[w][:, cs])
                    return t

                wts = {}

                def load_w(oc):
                    wt = wpool.tile([128, 32 * 128], F16, tag=f"w{oc}",
                                    name=f"wt{oc}")
                    nc.sync.dma_start(wt[:], wqkv_d[oc])
                    wts[oc] = wt

                load_w(K_OCS[0])  # first MMs need it
                ht = ht_load(0, 4)
                for oc in K_OCS[1:] + Q_OCS:
                    load_w(oc)
                wv_sb = wpool.tile([128, 32 * 512], F16, name="wv")
                nc.sync.dma_start(wv_sb[:], wv_d[:])
                nc.sync.dma_start(rope_sb[:], rope_d[:])
                nc.sync.dma_start(rt_sb[:], rt_d[:])

                def project(wt, dest, rot, ws):
                    # dest: [128, 512] fp16 slice (stage tile or resident kt)
                    acc = apool.tile([128, 512], F32)
                    for hc in range(32):
                        nc.tensor.matmul(
                            acc[:], wt[:, hc * 128:(hc + 1) * 128],
                            ht[:, hc * 512:(hc + 1) * 512],
                            start=(hc == 0), stop=(hc == 31))
                    nc.scalar.copy(dest[:], acc[:])
                    if rot:
                        # partial rotary on first 64 dims of this head
                        rp = rpool.tile([64, 512], F32)
                        nc.tensor.matmul(rp[:], rt_sb[:], dest[0:64, :])
                        t1 = tpool.tile([64, 512], F16, tag="t1")
                        nc.vector.tensor_mul(
                            t1[:], acc[0:64, :], rope_sb[0:64, ws])
                        t2 = tpool.tile([64, 512], F16, tag="t2")
                        nc.vector.tensor_mul(
                            t2[:], rp[:], rope_sb[64:128, ws])
                        nc.vector.tensor_add(dest[0:64, :], t1[:], t2[:])

                for w in range(8):
                    if w > 0:
                        ht = next_ht
                    b, wo = w // 4, (w % 4) * 512
                    ws = slice(w * 512, (w + 1) * 512)
                    for j, oc in enumerate(K_OCS):
                        if j == 1 and w < 7:
                            # prefetch next window under this one's compute
                            next_ht = ht_load(w + 1, 2 if w < 2 else 1)
                        rot = oc in (4, 6)
                        if b == 0:
                            hl, dc = (oc - 4) // 2, (oc - 4) % 2
                            project(wts[oc], kts[(hl, dc)][:, wo:wo + 512],
                                    rot, ws)
                        else:
                            stage = spool.tile([128, 512], F16)
                            project(wts[oc], stage, rot, ws)
                            nc.sync.dma_start(
                                qkvT[(oc, b)][:, wo:wo + 512], stage[:])
                    for oc in Q_OCS:
                        stage = spool.tile([128, 512], F16)
                        project(wts[oc], stage, oc in (0, 2), ws)
                        nc.sync.dma_start(
                            qkvT[(oc, b)][:, wo:wo + 512], stage[:])
                    # v projection: hidden chunks stationary, w_v moving;
                    # output lands directly as [token, vdim]
                    for tc_ in range(4):
                        acc = apool.tile([128, 512], F32)
                        for hc in range(32):
                            nc.tensor.matmul(
                                acc[:],
                                ht[:, hc * 512 + tc_ * 128:
                                   hc * 512 + (tc_ + 1) * 128],
                                wv_sb[:, hc * 512:(hc + 1) * 512],
                                start=(hc == 0), stop=(hc == 31))
                        stage = spool.tile([128, 512], F16)
                        nc.scalar.copy(stage[:], acc[:])
                        kc = (w % 4) * 4 + tc_
                        nc.sync.dma_start(
                            vh_dram[b][:, kc * 512:(kc + 1) * 512], stage[:])

            # ---------------- Phase 2: attention + out-proj ----------------
            with ExitStack() as st2:
                ec = st2.enter_context
                c3 = ec(tc.tile_pool(name="p2w", bufs=1))
                vhpool = ec(tc.tile_pool(name="vh", bufs=1))
                qpool = ec(tc.tile_pool(name="qq", bufs=2))
                expool = ec(tc.tile_pool(name="ex", bufs=6))
                dnpool = ec(tc.tile_pool(name="dna", bufs=2))
                recpool = ec(tc.tile_pool(name="rec", bufs=2))
                aopool = ec(tc.tile_pool(name="ao", bufs=2))
                ospool = ec(tc.tile_pool(name="os", bufs=4))
                scpool = ec(tc.tile_pool(name="sc", bufs=2, space="PSUM"))
                avpool = ec(tc.tile_pool(name="av", bufs=1, space="PSUM"))
                dppool = ec(tc.tile_pool(name="dp", bufs=2, space="PSUM"))
                oppool = ec(tc.tile_pool(name="op", bufs=2, space="PSUM"))
                wout_sb = c3.tile([128, 4, H], F16)
                msk_sb = c3.tile([128, 4, 512], F16)
                nc.sync.dma_start(msk_sb[:], msk_d[:])
                kb_sb = c3.tile([128, 32], F32)
                nc.sync.dma_start(kb_sb[:], kb_d[:])
                onm_sb = c3.tile([128, 128], F32)
                nc.sync.dma_start(onm_sb[:], onm_d[:])

                def emit_outproj(b, qt, aos):
                    qo = qt * 512
                    for tc_ in range(4):
                        for ht_ in range(8):
                            op = oppool.tile([128, 512], F32, tag="op")
                            for ci, (hl, dc) in enumerate(
                                    ((0, 0), (0, 1), (1, 0), (1, 1))):
                                nc.tensor.matmul(
                                    op[:],
                                    aos[(hl, dc)][:, tc_ * 128:(tc_ + 1) * 128],
                                    wout_sb[:, 2 * hl + dc,
                                            ht_ * 512:(ht_ + 1) * 512],
                                    start=(ci == 0), stop=(ci == 3))
                            os_ = ospool.tile([128, 512], F16)
                            # split PSUM evacuation across both copy engines
                            if ht_ % 2 == 0:
                                nc.scalar.copy(os_[:], op[:])
                            else:
                                nc.vector.tensor_copy(os_[:], op[:])
                            r0 = b * 2048 + qo + tc_ * 128
                            nc.sync.dma_start(
                                out_d[r0:r0 + 128, ht_ * 512:(ht_ + 1) * 512],
                                os_[:])

                pending = None
                for b in range(2):
                    vh = vhpool.tile([128, 16 * 512], F16, tag="vha")
                    nc.sync.dma_start(vh[:], vh_dram[b][:])
                    if b == 1:
                        for hl in range(2):
                            for dc in range(2):
                                nc.sync.dma_start(
                                    kts[(hl, dc)][:],
                                    qkvT[(4 + 2 * hl + dc, b)][:])
                    for qt in range(4):
                        nkc = 4 * qt + 4  # causal: k-chunks beyond are all-masked
                        qo = qt * 512
                        aos = {}
                        for hl in range(2):
                            qs = []
                            for dc in range(2):
                                q = qpool.tile([128, 512], F16, tag=f"q{dc}")
                                nc.sync.dma_start(
                                    q[:], qkvT[(2 * hl + dc, b)][:, qo:qo + 512])
                                qs.append(q)
                            if b == 0 and qt == 0 and hl == 0:
                                # out-proj weights stream in column chunks so
                                # the first out-proj block only waits on 2MB
                                for ch in range(8):
                                    cs = slice(ch * 512, (ch + 1) * 512)
                                    nc.sync.dma_start(
                                        wout_sb[:, :, cs], wout_d[:, :, cs])
                            av0 = avpool.tile([128, 512], F32, tag="av0")
                            av1 = avpool.tile([128, 512], F32, tag="av1")
                            den_acc = dnpool.tile([128, 512], F32, tag="dna")

                            def emit_av(kc, ex):
                                nc.tensor.matmul(
                                    av0[:],
                                    vh[:, kc * 512 + hl * 256:
                                       kc * 512 + hl * 256 + 128],
                                    ex[:], start=(kc == 0), stop=(kc == nkc - 1))
                                nc.tensor.matmul(
                                    av1[:],
                                    vh[:, kc * 512 + hl * 256 + 128:
                                       kc * 512 + hl * 256 + 256],
                                    ex[:], start=(kc == 0), stop=(kc == nkc - 1))

                            prev = None
                            for kc in range(nkc):
                                sc = scpool.tile([128, 512], F32)
                                nc.tensor.matmul(
                                    sc[:], kts[(hl, 0)][:, kc * 128:(kc + 1) * 128],
                                    qs[0][:], start=True, stop=False)
                                nc.tensor.matmul(
                                    sc[:], kts[(hl, 1)][:, kc * 128:(kc + 1) * 128],
                                    qs[1][:], start=False, stop=True)
                                if kc >= 4 * qt:
                                    nc.vector.tensor_add(
                                        sc[:], sc[:], msk_sb[:, kc - 4 * qt, :])
                                # software pipeline: AV for the PREVIOUS chunk
                                # is emitted after this chunk's score matmuls,
                                # hiding the exp latency from the PE stream
                                if prev is not None:
                                    emit_av(*prev)
                                ex = expool.tile([128, 512], F16)
                                nc.scalar.activation(
                                    ex[:], sc[:], AF.Exp, scale=1.0 / 16.0,
                                    bias=kb_sb[:, b * 16 + kc:b * 16 + kc + 1])
                                # softmax denominator: per-partition partial
                                # sums accumulate on VectorE, freeing the PE
                                # of the per-chunk ones-matmul
                                if kc == 0:
                                    nc.vector.tensor_copy(den_acc[:], ex[:])
                                else:
                                    nc.vector.tensor_add(
                                        den_acc[:], den_acc[:], ex[:])
                                prev = (kc, ex)
                            emit_av(*prev)
                            # fast av-bank evacuation on ScalarE (DVE's in-order
                            # queue runs the ~4us reciprocal)
                            avs = []
                            for dc, av in ((0, av0), (1, av1)):
                                avc = aopool.tile([128, 512], F32, bufs=1,
                                                  tag=f"avs{hl}{dc}", name="avc")
                                nc.scalar.copy(avc[:], av[:])
                                avs.append(avc)
                            # partition-reduce the denominator partials
                            # (f32 matmul, pre-broadcast across partitions)
                            den = dppool.tile([128, 512], F32)
                            nc.tensor.matmul(den[:], onm_sb[:], den_acc[:])
                            rec = recpool.tile([128, 512], F32, tag="rec", bufs=1)
                            nc.vector.reciprocal(rec[:], den[:])
                            for dc in range(2):
                                ao = aopool.tile([128, 512], F16, tag=f"ao{hl}{dc}")
                                nc.vector.tensor_mul(ao[:], avs[dc][:], rec[:])
                                aos[(hl, dc)] = ao
                        # software pipeline: emit the PREVIOUS block's out-proj
                        # here so its matmuls sit behind this block's attention
                        # in PE program order and never wait on normalization
                        if pending is not None:
                            emit_outproj(*pending)
                        pending = (b, qt, aos)
                emit_outproj(*pending)
    nc.compile()
    return nc


def _get_nc():
    if not _NC_CACHE:
        _NC_CACHE.append(_build())
    return _NC_CACHE[0]


def _host_prep(hidden_states, position_ids, attention_mask, w_qkv, w_out):
    hid = np.ascontiguousarray(np.asarray(hidden_states, np.float32)).reshape(TOK, H)
    w_qkv = np.asarray(w_qkv, np.float32)
    w_out = np.asarray(w_out, np.float32)
    pos = np.asarray(position_ids).astype(np.int64)
    am = np.asarray(attention_mask).reshape(B, S).astype(bool)

    # hsT window tiles [w, p, hc*512+t]
    hst = np.ascontiguousarray(
        hid.reshape(8, 512, 32, 128).transpose(0, 3, 2, 1)
    ).reshape(8, 128, 32 * 512).astype(np.float16)

    # rotary tables, matching reference.create_sinusoidal_positions
    inv_freq = 1.0 / 10000 ** (np.arange(0, ROT, 2) / ROT)
    si = np.einsum('i,j->ij', np.arange(MAX_POS), inv_freq).astype('float32')
    emb = np.concatenate([np.sin(si), np.cos(si)], axis=-1)  # [2048, 64]
    sincos = emb[pos]                    # [B, S, 64]
    sin_rep = np.repeat(sincos[..., :ROT // 2], 2, axis=2)   # [B, S, 64]
    cos_rep = np.repeat(sincos[..., ROT // 2:], 2, axis=2)
    rope = np.empty((128, TOK), np.float16)
    rope[0:64] = cos_rep.reshape(TOK, 64).T
    rope[64:128] = sin_rep.reshape(TOK, 64).T

    rt = np.zeros((64, 64), np.float16)
    rt[np.arange(1, 64, 2), np.arange(0, 64, 2)] = -1.0
    rt[np.arange(0, 64, 2), np.arange(1, 64, 2)] = 1.0

    onesm = np.ones((128, 128), np.float32)

    p_idx = np.arange(128)[:, None, None]
    i_idx = np.arange(4)[None, :, None]
    q_idx = np.arange(512)[None, None, :]
    masks = np.where(p_idx + i_idx * 128 <= q_idx, 0.0, NEG).astype(np.float16)

    kb = np.where(am.reshape(B, 16, 128), 0.0, NEG).astype(
        np.float32).transpose(2, 0, 1).reshape(128, 32)
    kb = np.ascontiguousarray(kb)

    shared = dict(hst=hst, rope=rope, rt=rt, onesm=onesm, masks=masks, kb=kb)

    in_maps = []
    for c in range(N_CORES):
        qk_cols = []
        v_cols = []
        # fused layout per mp-group is (query, value, key)
        for part, dest in ((0, qk_cols), (2, qk_cols), (1, v_cols)):
            for hl in range(HPC):
                h = HPC * c + hl
                base = (h // 4) * 3072 + part * 1024 + (h % 4) * 256
                dest.append(np.arange(base, base + 256))
        qk_cols = np.concatenate(qk_cols)  # [1024] = q(512) | k(512)
        wslice = w_qkv[:, qk_cols]         # [4096, 1024]
        wqkv_prep = np.ascontiguousarray(
            wslice.reshape(32, 128, 8, 128).transpose(2, 1, 0, 3)
        ).reshape(8, 128, 32 * 128).astype(np.float16)
        v_cols = np.concatenate(v_cols)    # [512]
        wv_prep = np.ascontiguousarray(
            w_qkv[:, v_cols].reshape(32, 128, 512).transpose(1, 0, 2)
        ).reshape(128, 32 * 512).astype(np.float16)
        wout_prep = np.ascontiguousarray(
            w_out[c * DPC:(c + 1) * DPC, :].reshape(4, 128, H).transpose(1, 0, 2)
        ).astype(np.float16)
        in_maps.append(dict(shared, wqkv=wqkv_prep, wv=wv_prep, wout=wout_prep))
    return in_maps


def kernel(hidden_states, position_ids, attention_mask, w_qkv, w_out):
    global LAST_EXEC_NS
    nc = _get_nc()
    in_maps = _host_prep(hidden_states, position_ids, attention_mask,
                         w_qkv, w_out)
    res = run_bass_kernel_spmd(nc, in_maps, core_ids=list(range(N_CORES)))
    LAST_EXEC_NS = res.exec_time_ns
    out = res.results[0]["out"].astype(np.float32)
    for c in range(1, N_CORES):
        out = out + res.results[c]["out"].astype(np.float32)
    return out.reshape(B, S, H)


# revision 4
# speedup vs baseline: 1.4299x; 1.0123x over previous
"""CodeGen-style attention block, tensor-parallel over heads on 8 Trainium2 cores.

Strategy (megatron-style):
  - Each core owns 2 of the 16 heads: computes Q/K/V projections for its
    head-slice of w_qkv, runs causal attention for those heads, then applies
    its row-slice of w_out, producing a partial [tokens, H] output.
  - Host sums the 8 partial outputs (the out-proj contraction over heads).

v4 notes:
  - All matmuls are fp16 (fp32 PSUM accumulate). On HW, fp32r moving operands
    stream at ~0.55 ns/col (bandwidth-limited) and fp32 LDWEIGHTS skips FWL;
    fp16 streams at the full 1 col/cycle rate and halves every DMA transfer.
  - Single pass over the hidden states: all 12 w_qkv column-chunks stay
    resident in SBUF, so hidden is read once.
  - V is projected directly into [token, dim] layout (hidden chunks
    stationary, w_v moving) - no PE transposes / DVE copies in phase 2.
  - K for batch 0 is written straight into resident SBUF tiles by phase 1
    (no DRAM round-trip).
  - All stores (projection spills, output writes) issue on the GpSimd SWDGE
    queue; the Sync HWDGE queue carries only loads, so prefetches are never
    stuck behind bulk writes.
  - Attention inner loop is software-pipelined: scores for chunk kc+1 are
    issued before AV of chunk kc, so the PE never waits on ScalarE's exp.
    The softmax denominator rides the PE as one fp16 ones-matmul per chunk
    (accumulating in PSUM) - cheaper than any cross-engine scheme in wall
    time because it adds no cross-engine hop to the block tail.
  - Diagonal k-chunks only compute the live column subrange [i*128, 512) -
    the masked prefix contributes exactly zero and is skipped in the score/
    exp/AV/denominator ops.
  - Causal block-skipping: score/AV work for fully-masked k-chunks is skipped.
"""

import sys
import types
from contextlib import ExitStack

import numpy as np

import concourse.bacc as bacc
import concourse.mybir as mybir
import concourse.tile as tile
from concourse.bass_utils import run_bass_kernel_spmd

# bass_utils imports antenv.axon_hooks when tracing is requested via env;
# provide a no-op stub if the module is absent so a stray BASS_TRACE in the
# environment cannot break execution.
try:
    import antenv.axon_hooks  # noqa: F401
except ImportError:
    _stub = types.ModuleType("antenv.axon_hooks")
    _stub.get_axon_ntff_profile_hook = lambda: None
    _stub.set_axon_ntff_profile_hook = lambda h: None
    sys.modules.setdefault("antenv.axon_hooks", _stub)

F32 = mybir.dt.float32
F16 = mybir.dt.float16
AF = mybir.ActivationFunctionType

B, S, H = 2, 2048, 4096
N_HEAD, HEAD_DIM, ROT = 16, 256, 64
MAX_POS = 2048
TOK = B * S            # 4096
N_CORES = 8
HPC = N_HEAD // N_CORES  # heads per core = 2
DPC = HPC * HEAD_DIM     # dims per core = 512
NEG = -30000.0

LAST_EXEC_NS = None
_NC_CACHE = []


def _build():
    nc = bacc.Bacc("TRN2", target_bir_lowering=False, debug=False,
                   num_devices=N_CORES)

    # [w, p, hc*512+t]: hsT window tiles (512 tokens each), per-partition-contiguous
    hst_d = nc.dram_tensor("hst", [8, 128, 32 * 512], F16, kind="ExternalInput")
    # [oc, p, hc*128+d]: per-core w_qkv column-chunks for q (oc 0-3), k (oc 4-7)
    wqkv_d = nc.dram_tensor("wqkv", [8, 128, 32 * 128], F16, kind="ExternalInput")
    # [p, hc*512+v]: per-core w_v slice, hidden-chunk-major (moving operand)
    wv_d = nc.dram_tensor("wv", [128, 32 * 512], F16, kind="ExternalInput")
    # [p, c, n]: per-core w_out row-slice
    wout_d = nc.dram_tensor("wout", [128, 4, H], F16, kind="ExternalInput")
    rope_d = nc.dram_tensor("rope", [128, TOK], F16, kind="ExternalInput")
    rt_d = nc.dram_tensor("rt", [64, 64], F16, kind="ExternalInput")
    onm_d = nc.dram_tensor("onesm", [128, 128], F16, kind="ExternalInput")
    msk_d = nc.dram_tensor("masks", [128, 4, 128], F16, kind="ExternalInput")
    kb_d = nc.dram_tensor("kb", [128, 32], F32, kind="ExternalInput")
    out_d = nc.dram_tensor("out", [TOK, H], F16, kind="ExternalOutput")

    # per-window oc order: k first (its outputs land in resident SBUF tiles
    # for b=0 and phase 2 reads k/v before q), then q, then v.
    K_OCS = (4, 5, 6, 7)
    Q_OCS = (0, 1, 2, 3)

    with tile.TileContext(nc) as tc:
        with ExitStack() as st0:
            ec0 = st0.enter_context
            dram_pool = ec0(tc.tile_pool(name="dram", bufs=1, space="DRAM"))
            # per-(oc, batch) intermediates so phase-2 loads only wait on the
            # phase-1 windows they actually read. b=0 k skips DRAM entirely.
            qkvT = {}
            for oc in range(8):
                for b in range(2):
                    if oc >= 4 and b == 0:
                        continue
                    qkvT[(oc, b)] = dram_pool.tile(
                        [128, 2048], F16, tag=f"qkvT{oc}_{b}",
                        name=f"qkvT{oc}_{b}")
            vh_dram = {}
            for b in range(2):
                vh_dram[b] = dram_pool.tile(
                    [128, 16 * 512], F16, tag=f"vh{b}", name=f"vh{b}")
            # resident k tiles: phase 1 writes b=0 directly, phase 2 reloads
            # them from DRAM for b=1
            kpool = ec0(tc.tile_pool(name="kt", bufs=1))
            kts = {}
            for hl in range(2):
                for dc in range(2):
                    kts[(hl, dc)] = kpool.tile(
                        [128, 2048], F16, tag=f"kt{hl}{dc}", name=f"kt{hl}{dc}")

            # ---------------- Phase 1: QKV projection + rotary ----------------
            with ExitStack() as st1:
                ec = st1.enter_context
                cpool = ec(tc.tile_pool(name="p1c", bufs=1))
                wpool = ec(tc.tile_pool(name="w", bufs=1))
                hpool = ec(tc.tile_pool(name="ht", bufs=2))
                spool = ec(tc.tile_pool(name="stage", bufs=4))
                tpool = ec(tc.tile_pool(name="rott", bufs=4))
                apool = ec(tc.tile_pool(name="acc", bufs=4, space="PSUM"))
                rpool = ec(tc.tile_pool(name="rp", bufs=2, space="PSUM"))
                rope_sb = cpool.tile([128, TOK], F16)
                rt_sb = cpool.tile([64, 64], F16)

                def ht_load(w, strips):
                    # strip the transfer so the first H-chunks land (and the
                    # first matmuls start) before the whole 4MB tile arrives
                    t = hpool.tile([128, 32 * 512], F16, name="ht")
                    step = 32 // strips
                    for s in range(strips):
                        cs = slice(s * step * 512, (s + 1) * step * 512)
                        nc.sync.dma_start(t[:, cs], hst_d[w][:, cs])
                    return t

                wts = {}

                def load_w(oc):
                    wt = wpool.tile([128, 32 * 128], F16, tag=f"w{oc}",
                                    name=f"wt{oc}")
                    nc.sync.dma_start(wt[:], wqkv_d[oc])
                    wts[oc] = wt

                load_w(K_OCS[0])  # first MMs need it
                ht = ht_load(0, 4)
                for oc in K_OCS[1:] + Q_OCS:
                    load_w(oc)
                wv_sb = wpool.tile([128, 32 * 512], F16, name="wv")
                nc.sync.dma_start(wv_sb[:], wv_d[:])
                nc.sync.dma_start(rope_sb[:], rope_d[:])
                nc.sync.dma_start(rt_sb[:], rt_d[:])

                def project(wt, dest, rot, ws):
                    # dest: [128, 512] fp16 slice (stage tile or resident kt)
                    acc = apool.tile([128, 512], F32)
                    for hc in range(32):
                        nc.tensor.matmul(
                            acc[:], wt[:, hc * 128:(hc + 1) * 128],
                            ht[:, hc * 512:(hc + 1) * 512],
                            start=(hc == 0), stop=(hc == 31))
                    nc.scalar.copy(dest[:], acc[:])
                    if rot:
                        # partial rotary on first 64 dims of this head
                        rp = rpool.tile([64, 512], F32)
                        nc.tensor.matmul(rp[:], rt_sb[:], dest[0:64, :])
                        t1 = tpool.tile([64, 512], F16, tag="t1")
                        nc.vector.tensor_mul(
                            t1[:], acc[0:64, :], rope_sb[0:64, ws])
                        t2 = tpool.tile([64, 512], F16, tag="t2")
                        nc.vector.tensor_mul(
                            t2[:], rp[:], rope_sb[64:128, ws])
                        nc.vector.tensor_add(dest[0:64, :], t1[:], t2[:])

                for w in range(8):
                    if w > 0:
                        ht = next_ht
                    b, wo = w // 4, (w % 4) * 512
                    ws = slice(w * 512, (w + 1) * 512)
                    for j, oc in enumerate(K_OCS):
                        if j == 1 and w < 7:
                            # prefetch next window under this one's compute
                            next_ht = ht_load(w + 1, 2 if w < 2 else 1)
                        rot = oc in (4, 6)
                        if b == 0:
                            hl, dc = (oc - 4) // 2, (oc - 4) % 2
                            project(wts[oc], kts[(hl, dc)][:, wo:wo + 512],
                                    rot, ws)
                        else:
                            stage = spool.tile([128, 512], F16)
                            project(wts[oc], stage, rot, ws)
                            nc.gpsimd.dma_start(
                                qkvT[(oc, b)][:, wo:wo + 512], stage[:])
                    for oc in Q_OCS:
                        stage = spool.tile([128, 512], F16)
                        project(wts[oc], stage, oc in (0, 2), ws)
                        nc.gpsimd.dma_start(
                            qkvT[(oc, b)][:, wo:wo + 512], stage[:])
                    # v projection: hidden chunks stationary, w_v moving;
                    # output lands directly as [token, vdim]
                    for tc_ in range(4):
                        acc = apool.tile([128, 512], F32)
                        for hc in range(32):
                            nc.tensor.matmul(
                                acc[:],
                                ht[:, hc * 512 + tc_ * 128:
                                   hc * 512 + (tc_ + 1) * 128],
                                wv_sb[:, hc * 512:(hc + 1) * 512],
                                start=(hc == 0), stop=(hc == 31))
                        stage = spool.tile([128, 512], F16)
                        nc.scalar.copy(stage[:], acc[:])
                        kc = (w % 4) * 4 + tc_
                        nc.gpsimd.dma_start(
                            vh_dram[b][:, kc * 512:(kc + 1) * 512], stage[:])

            # ---------------- Phase 2: attention + out-proj ----------------
            with ExitStack() as st2:
                ec = st2.enter_context
                c3 = ec(tc.tile_pool(name="p2w", bufs=1))
                vhpool = ec(tc.tile_pool(name="vh", bufs=1))
                qpool = ec(tc.tile_pool(name="qq", bufs=2))
                expool = ec(tc.tile_pool(name="ex", bufs=6))
                recpool = ec(tc.tile_pool(name="rec", bufs=2))
                aopool = ec(tc.tile_pool(name="ao", bufs=2))
                ospool = ec(tc.tile_pool(name="os", bufs=4))
                scpool = ec(tc.tile_pool(name="sc", bufs=2, space="PSUM"))
                avpool = ec(tc.tile_pool(name="av", bufs=1, space="PSUM"))
                denpool = ec(tc.tile_pool(name="den", bufs=2, space="PSUM"))
                oppool = ec(tc.tile_pool(name="op", bufs=2, space="PSUM"))
                wout_sb = c3.tile([128, 4, H], F16)
                msk_sb = c3.tile([128, 4, 128], F16)
                nc.sync.dma_start(msk_sb[:], msk_d[:])
                kb_sb = c3.tile([128, 32], F32)
                nc.sync.dma_start(kb_sb[:], kb_d[:])
                onm_sb = c3.tile([128, 128], F16)
                nc.sync.dma_start(onm_sb[:], onm_d[:])

                def emit_outproj(b, qt, aos):
                    qo = qt * 512
                    for tc_ in range(4):
                        for ht_ in range(8):
                            op = oppool.tile([128, 512], F32, tag="op")
                            for ci, (hl, dc) in enumerate(
                                    ((0, 0), (0, 1), (1, 0), (1, 1))):
                                nc.tensor.matmul(
                                    op[:],
                                    aos[(hl, dc)][:, tc_ * 128:(tc_ + 1) * 128],
                                    wout_sb[:, 2 * hl + dc,
                                            ht_ * 512:(ht_ + 1) * 512],
                                    start=(ci == 0), stop=(ci == 3))
                            os_ = ospool.tile([128, 512], F16)
                            # split PSUM evacuation across both copy engines
                            if ht_ % 2 == 0:
                                nc.scalar.copy(os_[:], op[:])
                            else:
                                nc.vector.tensor_copy(os_[:], op[:])
                            r0 = b * 2048 + qo + tc_ * 128
                            nc.gpsimd.dma_start(
                                out_d[r0:r0 + 128, ht_ * 512:(ht_ + 1) * 512],
                                os_[:])

                pending = None
                for b in range(2):
                    vh = vhpool.tile([128, 16 * 512], F16, tag="vha")
                    nc.sync.dma_start(vh[:], vh_dram[b][:])
                    if b == 1:
                        for hl in range(2):
                            for dc in range(2):
                                nc.sync.dma_start(
                                    kts[(hl, dc)][:],
                                    qkvT[(4 + 2 * hl + dc, b)][:])
                    for qt in range(4):
                        nkc = 4 * qt + 4  # causal: k-chunks beyond are all-masked
                        qo = qt * 512
                        # all four q tiles up-front: loads ride the (store-free)
                        # sync queue and land well before the PE needs them
                        qmap = {}
                        for hl in range(2):
                            for dc in range(2):
                                q = qpool.tile([128, 512], F16, tag=f"q{hl}{dc}")
                                nc.sync.dma_start(
                                    q[:], qkvT[(2 * hl + dc, b)][:, qo:qo + 512])
                                qmap[(hl, dc)] = q
                        if b == 0 and qt == 0:
                            # out-proj weights stream in column chunks so the
                            # first out-proj block only waits on 2MB
                            for ch in range(8):
                                cs = slice(ch * 512, (ch + 1) * 512)
                                nc.sync.dma_start(
                                    wout_sb[:, :, cs], wout_d[:, :, cs])
                        aos = {}
                        for hl in range(2):
                            qs = [qmap[(hl, 0)], qmap[(hl, 1)]]
                            av0 = avpool.tile([128, 512], F32, tag="av0")
                            av1 = avpool.tile([128, 512], F32, tag="av1")
                            den = denpool.tile([128, 512], F32)

                            def emit_av(kc, ex, lo):
                                nc.tensor.matmul(
                                    av0[:, lo:512],
                                    vh[:, kc * 512 + hl * 256:
                                       kc * 512 + hl * 256 + 128],
                                    ex[:, lo:512],
                                    start=(kc == 0), stop=(kc == nkc - 1))
                                nc.tensor.matmul(
                                    av1[:, lo:512],
                                    vh[:, kc * 512 + hl * 256 + 128:
                                       kc * 512 + hl * 256 + 256],
                                    ex[:, lo:512],
                                    start=(kc == 0), stop=(kc == nkc - 1))
                                # denominator, pre-broadcast across partitions:
                                # ones[128,128].T @ ex = colsum replicated 128x
                                nc.tensor.matmul(
                                    den[:, lo:512], onm_sb[:], ex[:, lo:512],
                                    start=(kc == 0), stop=(kc == nkc - 1))

                            prev = None
                            for kc in range(nkc):
                                # diagonal chunks: columns below lo are fully
                                # masked (zero contribution) and are skipped
                                lo = max(0, (kc - 4 * qt) * 128)
                                sc = scpool.tile([128, 512], F32)
                                nc.tensor.matmul(
                                    sc[:, lo:512],
                                    kts[(hl, 0)][:, kc * 128:(kc + 1) * 128],
                                    qs[0][:, lo:512], start=True, stop=False)
                                nc.tensor.matmul(
                                    sc[:, lo:512],
                                    kts[(hl, 1)][:, kc * 128:(kc + 1) * 128],
                                    qs[1][:, lo:512], start=False, stop=True)
                                if kc >= 4 * qt:
                                    # triangular mask only touches the 128-wide
                                    # diagonal strip
                                    nc.vector.tensor_add(
                                        sc[:, lo:lo + 128], sc[:, lo:lo + 128],
                                        msk_sb[:, kc - 4 * qt, :])
                                # software pipeline: AV for the PREVIOUS chunk
                                # is emitted after this chunk's score matmuls,
                                # hiding the exp latency from the PE stream
                                if prev is not None:
                                    emit_av(*prev)
                                ex = expool.tile([128, 512], F16)
                                nc.scalar.activation(
                                    ex[:, lo:512], sc[:, lo:512], AF.Exp,
                                    scale=1.0 / 16.0,
                                    bias=kb_sb[:, b * 16 + kc:b * 16 + kc + 1])
                                prev = (kc, ex, lo)
                            emit_av(*prev)
                            # av-bank evacuation split across ScalarE and DVE so
                            # the banks free ~720ns after the last AV matmul and
                            # the next block's first AV never waits
                            avc0 = aopool.tile([128, 512], F32, bufs=1,
                                               tag=f"avs{hl}0", name="avc")
                            nc.scalar.copy(avc0[:], av0[:])
                            avc1 = aopool.tile([128, 512], F32, bufs=1,
                                               tag=f"avs{hl}1", name="avc")
                            nc.vector.tensor_copy(avc1[:], av1[:])
                            rec = recpool.tile([128, 512], F32, tag="rec", bufs=1)
                            nc.vector.reciprocal(rec[:], den[:])
                            for dc, avc in ((0, avc0), (1, avc1)):
                                ao = aopool.tile([128, 512], F16, tag=f"ao{hl}{dc}")
                                nc.vector.tensor_mul(ao[:], avc[:], rec[:])
                                aos[(hl, dc)] = ao
                        # software pipeline: emit the PREVIOUS block's out-proj
                        # here so its matmuls sit behind this block's attention
                        # in PE program order and never wait on normalization
                        if pending is not None:
                            emit_outproj(*pending)
                        pending = (b, qt, aos)
                emit_outproj(*pending)
    nc.compile()
    return nc


def _get_nc():
    if not _NC_CACHE:
        _NC_CACHE.append(_build())
    return _NC_CACHE[0]


def _host_prep(hidden_states, position_ids, attention_mask, w_qkv, w_out):
    hid = np.ascontiguousarray(np.asarray(hidden_states, np.float32)).reshape(TOK, H)
    w_qkv = np.asarray(w_qkv, np.float32)
    w_out = np.asarray(w_out, np.float32)
    pos = np.asarray(position_ids).astype(np.int64)
    am = np.asarray(attention_mask).reshape(B, S).astype(bool)

    # hsT window tiles [w, p, hc*512+t]
    hst = np.ascontiguousarray(
        hid.reshape(8, 512, 32, 128).transpose(0, 3, 2, 1)
    ).reshape(8, 128, 32 * 512).astype(np.float16)

    # rotary tables, matching reference.create_sinusoidal_positions
    inv_freq = 1.0 / 10000 ** (np.arange(0, ROT, 2) / ROT)
    si = np.einsum('i,j->ij', np.arange(MAX_POS), inv_freq).astype('float32')
    emb = np.concatenate([np.sin(si), np.cos(si)], axis=-1)  # [2048, 64]
    sincos = emb[pos]                    # [B, S, 64]
    sin_rep = np.repeat(sincos[..., :ROT // 2], 2, axis=2)   # [B, S, 64]
    cos_rep = np.repeat(sincos[..., ROT // 2:], 2, axis=2)
    rope = np.empty((128, TOK), np.float16)
    rope[0:64] = cos_rep.reshape(TOK, 64).T
    rope[64:128] = sin_rep.reshape(TOK, 64).T

    rt = np.zeros((64, 64), np.float16)
    rt[np.arange(1, 64, 2), np.arange(0, 64, 2)] = -1.0
    rt[np.arange(0, 64, 2), np.arange(1, 64, 2)] = 1.0

    onesm = np.ones((128, 128), np.float16)

    # triangular mask for the 128-wide diagonal strip of each diagonal chunk:
    # key partition p (within chunk i) vs query column q0+lo..lo+127
    p_idx = np.arange(128)[:, None, None]
    i_idx = np.arange(4)[None, :, None]
    q_idx = np.arange(128)[None, None, :]
    masks = np.where(p_idx <= q_idx + 0 * i_idx, 0.0, NEG).astype(np.float16)

    kb = np.where(am.reshape(B, 16, 128), 0.0, NEG).astype(
        np.float32).transpose(2, 0, 1).reshape(128, 32)
    kb = np.ascontiguousarray(kb)

    shared = dict(hst=hst, rope=rope, rt=rt, onesm=onesm, masks=masks, kb=kb)

    in_maps = []
    for c in range(N_CORES):
        qk_cols = []
        v_cols = []
        # fused layout per mp-group is (query, value, key)
        for part, dest in ((0, qk_cols), (2, qk_cols), (1, v_cols)):
            for hl in range(HPC):
                h = HPC * c + hl
                base = (h // 4) * 3072 + part * 1024 + (h % 4) * 256
                dest.append(np.arange(base, base + 256))
        qk_cols = np.concatenate(qk_cols)  # [1024] = q(512) | k(512)
        wslice = w_qkv[:, qk_cols]         # [4096, 1024]
        wqkv_prep = np.ascontiguousarray(
            wslice.reshape(32, 128, 8, 128).transpose(2, 1, 0, 3)
        ).reshape(8, 128, 32 * 128).astype(np.float16)
        v_cols = np.concatenate(v_cols)    # [512]
        wv_prep = np.ascontiguousarray(
            w_qkv[:, v_cols].reshape(32, 128, 512).transpose(1, 0, 2)
        ).reshape(128, 32 * 512).astype(np.float16)
        wout_prep = np.ascontiguousarray(
            w_out[c * DPC:(c + 1) * DPC, :].reshape(4, 128, H).transpose(1, 0, 2)
        ).astype(np.float16)
        in_maps.append(dict(shared, wqkv=wqkv_prep, wv=wv_prep, wout=wout_prep))
    return in_maps


def kernel(hidden_states, position_ids, attention_mask, w_qkv, w_out):
    global LAST_EXEC_NS
    nc = _get_nc()
    in_maps = _host_prep(hidden_states, position_ids, attention_mask,
                         w_qkv, w_out)
    res = run_bass_kernel_spmd(nc, in_maps, core_ids=list(range(N_CORES)))
    LAST_EXEC_NS = res.exec_time_ns
    out = res.results[0]["out"].astype(np.float32)
    for c in range(1, N_CORES):
        out = out + res.results[c]["out"].astype(np.float32)
    return out.reshape(B, S, H)
